# revision 14
# baseline (speedup 1.0000x reference)
"""MoE routing kernel for Trainium2 (8 NeuronCores, SPMD).

Math being implemented (faithful to the reference, including its quirks):
  logits = x @ gate_w + gate_b                  # [B,S,E]
  weights = softmax(logits, axis=1)             # softmax over the SEQUENCE axis
  top2 values/indices over experts; only experts 0 and 1 are ever evaluated
  (the reference loops `for ind in range(top_k)` and uses expert `ind`).
  out[t] = c0[t]*eo_0[t] + c1[t]*eo_1[t], where
  eo_e = softmax_D(gelu(x@w1[e]+b1[e]) @ w2[e] + b2[e]) and c_e[t] is the
  top-2 gate weight when expert e is in token t's top-2, else 0.

Sharding: routing + dispatch on host (0.4% of FLOPs). Only tokens whose
top-2 contains expert 0/1 are computed (~25% each). Cores 0-3 handle
expert 0's tokens, cores 4-7 expert 1's. Device computes p = exp(z)
unnormalized in feature-major layout; the softmax division and gate-weight
scaling happen on host during the gather (O(T*D) adds, off the device
critical path).

Device kernel structure (per core, n tokens):
  - all of w1/w2 resident in SBUF (host pre-permuted layouts, ~30 large
    DMAs total); ring FIFO ordering delivers xs -> w1 groups -> w2 so
    phase B's weights never compete with phase A's
  - warmup matmuls on a memset tile so the PE HAM clock ramps to 2.4 GHz
    before real work lands
  - phase A: optionally the first `a_fp8_kd` k-tiles of the contraction run
    as fp8e4m3 DoubleRow matmuls (2x PE rate); the rest run fp16. m-major,
    k-inner, rotating PSUM banks, ACT applies gelu(+b1) -> h (fp16)
  - phase B: fp16; ACT exp(+b2) -> p, DMA'd out as each m-tile completes
"""

import sys

import numpy as np

sys.path.insert(0, "/opt/trn_rl_repo")

import concourse.bacc as bacc  # noqa: E402
import concourse.bass as bass  # noqa: E402
import concourse.tile as tile  # noqa: E402
from concourse import mybir  # noqa: E402
from concourse.bass_utils import run_bass_kernel_spmd  # noqa: E402

P = 128
D = 1024
F = 4096
NCORES = 8
CHUNK = 512  # psum bank free-dim capacity (f32)
MGC = 512    # w1 m-group column width (4 m-tiles)
A_FP8_KD = 4  # k-tiles (of 8) of phase A contraction in fp8 DoubleRow
AF = mybir.ActivationFunctionType

_CACHE = {}


def _gating_coeffs(x, gate_w, gate_b):
    """Host replica of the reference gating. Returns c[T,2] float32 where
    c[:,e] is the gate weight if expert e is in the token's top-2 else 0."""
    B, S, _ = x.shape
    x = np.asarray(x, dtype=np.float32)
    logits = x.reshape(B * S, -1) @ np.asarray(gate_w, dtype=np.float32)
    logits = logits.reshape(B, S, -1) + np.asarray(gate_b, dtype=np.float32)
    # softmax over the sequence axis (axis=1), as in the reference
    m = logits.max(axis=1, keepdims=True)
    e = np.exp(logits - m)
    w = e / e.sum(axis=1, keepdims=True)
    wf = w.reshape(B * S, -1)
    # stable argsort of -w == jax.lax.top_k tie semantics (lower index wins)
    top2 = np.argsort(-wf, axis=-1, kind="stable")[:, :2]
    c = np.zeros((B * S, 2), dtype=np.float32)
    for ex in (0, 1):
        sel = (top2 == ex).any(axis=1)
        c[sel, ex] = wf[sel, ex]
    return c


def _build_nc(n, a_fp8_kd, n_warm=12):
    """Bass program for one core: n tokens (multiple of 128), one expert."""
    dt = mybir.dt
    sdt = dt.float16
    f8 = dt.float8e4
    f32 = dt.float32
    chunks = []
    off = 0
    while off < n:
        sz = min(CHUNK, n - off)
        chunks.append((off, sz))
        off += sz
    KD, KF = D // P, F // P  # 8, 32
    MG = F // MGC            # 8 w1 column groups
    JG = MGC // P            # m-tiles per w1 group
    KK = a_fp8_kd // 2       # DoubleRow k-pairs
    K16 = KD - a_fp8_kd      # fp16 k-tiles in phase A
    DR = mybir.MatmulPerfMode.DoubleRow

    nc = bacc.Bacc()
    # host-prepermuted layouts (see kernel() below)
    if KK:
        x8d = nc.dram_tensor("xs8", [P, KK * 2, n], f8, kind="ExternalInput")
        w18d = nc.dram_tensor("w18", [P, MG * KK * 2, MGC], f8, kind="ExternalInput")
    xd = nc.dram_tensor("xs", [P, K16, n], sdt, kind="ExternalInput")
    w1d = nc.dram_tensor("w1p", [P, MG * K16, MGC], sdt, kind="ExternalInput")
    w2d = nc.dram_tensor("w2p", [P, KF * D], sdt, kind="ExternalInput")
    b1d = nc.dram_tensor("b1t", [P, KF], f32, kind="ExternalInput")
    b2d = nc.dram_tensor("b2t", [P, KD], f32, kind="ExternalInput")
    outT = nc.dram_tensor("outT", [D, n], sdt, kind="ExternalOutput")

    with tile.TileContext(nc) as tc:
        with (
            tc.tile_pool(name="const", bufs=1) as const,
            tc.tile_pool(name="acts", bufs=1) as acts,
            tc.tile_pool(name="ps", bufs=8, space="PSUM") as ps,
        ):
            # ---- input DMAs: few, large, spread across sequencers ----
            # Ring rates ~140 GB/s each on sync/gpsimd, aggregate ~285 GB/s.
            # FIFO per ring orders delivery: xs first (m=0 needs ALL k-tiles),
            # then w1 groups in consumption order, then w2 (needed only for
            # phase B, naturally deferred behind w1 by ring FIFO).
            xs16 = acts.tile([P, K16, n], sdt)
            if KK:
                xs8 = acts.tile([P, KK * 2, n], f8)
                nc.sync.dma_start(xs8[:, :, :], x8d[:, :, :])
                hk = K16 // 2
                nc.gpsimd.dma_start(xs16[:, :hk, :], xd[:, :hk, :])
                nc.sync.dma_start(xs16[:, hk:, :], xd[:, hk:, :])
            else:
                for q in range(4):
                    eng = nc.sync if q % 2 == 0 else nc.gpsimd
                    k0, k1 = q * K16 // 4, (q + 1) * K16 // 4
                    eng.dma_start(xs16[:, k0:k1, :], xd[:, k0:k1, :])
            b1t = const.tile([P, KF], f32)
            nc.scalar.dma_start(b1t[:], b1d[:])
            b2t = const.tile([P, KD], f32)
            nc.scalar.dma_start(b2t[:], b2d[:])

            if KK:
                w18 = acts.tile([P, MG * KK * 2, MGC], f8)
            w116 = acts.tile([P, MG * K16, MGC], sdt)
            for mg in range(MG):
                eng, eng2 = (nc.sync, nc.gpsimd) if mg % 2 == 0 else (nc.gpsimd, nc.sync)
                if mg == 0:
                    # m-tile 0's weight columns first: shrinks the critical
                    # DMA set gating the first real matmul
                    if KK:
                        eng.dma_start(w18[:, : KK * 2, :P], w18d[:, : KK * 2, :P])
                    eng2.dma_start(w116[:, :K16, :P], w1d[:, :K16, :P])
                    if KK:
                        eng2.dma_start(w18[:, : KK * 2, P:], w18d[:, : KK * 2, P:])
                    eng.dma_start(w116[:, :K16, P:], w1d[:, :K16, P:])
                    continue
                if KK:
                    eng.dma_start(
                        w18[:, mg * KK * 2 : (mg + 1) * KK * 2, :],
                        w18d[:, mg * KK * 2 : (mg + 1) * KK * 2, :],
                    )
                eng2.dma_start(
                    w116[:, mg * K16 : (mg + 1) * K16, :],
                    w1d[:, mg * K16 : (mg + 1) * K16, :],
                )
            w2s = acts.tile([P, KF * D], sdt)
            NW2 = 8
            for q in range(NW2):
                eng = nc.sync if q % 2 == 0 else nc.gpsimd
                sz = KF * D // NW2
                eng.dma_start(w2s[:, q * sz : (q + 1) * sz], w2d[:, q * sz : (q + 1) * sz])

            h = acts.tile([P, KF * n], sdt)
            p = acts.tile([P, KD * n], sdt)

            # ---- HAM warmup: matmuls gated only on a cheap DVE memset ----
            garb = const.tile([P, P + CHUNK], sdt)
            nc.vector.memset(garb[:], 1.0)
            warm_ps = ps.tile([P, CHUNK], f32, tag="ps", name="warm")
            for _ in range(n_warm):
                nc.tensor.matmul(
                    warm_ps[:], garb[:, :P], garb[:, P : P + CHUNK],
                    start=True, stop=True,
                )

            # ---- Phase A: h = gelu(w1.T @ x.T + b1), m-major k-inner ----
            for m in range(KF):
                mg, j = divmod(m, JG)
                for ci, (c0, csz) in enumerate(chunks):
                    acc = ps.tile([P, csz], f32, tag="ps", name=f"pa_{m}_{ci}")
                    for kk in range(KK):
                        i0 = (mg * KK + kk) * 2
                        nc.tensor.matmul(
                            acc[:],
                            w18[:, i0 : i0 + 2, j * P : (j + 1) * P],
                            xs8[:, 2 * kk : 2 * kk + 2, c0 : c0 + csz],
                            start=(kk == 0),
                            stop=False,
                            perf_mode=DR,
                        )
                    for k in range(K16):
                        nc.tensor.matmul(
                            acc[:],
                            w116[:, mg * K16 + k, j * P : (j + 1) * P],
                            xs16[:, k, c0 : c0 + csz],
                            start=(KK == 0 and k == 0),
                            stop=(k == K16 - 1),
                        )
                    nc.scalar.activation(
                        h[:, m * n + c0 : m * n + c0 + csz],
                        acc[:],
                        AF.Gelu,
                        bias=b1t[:, m : m + 1],
                    )

            # ---- Phase B: p = exp(w2.T @ h + b2); DMA out as computed ----
            for m in range(KD):
                for ci, (c0, csz) in enumerate(chunks):
                    acc = ps.tile([P, csz], f32, tag="ps", name=f"pb_{m}_{ci}")
                    for k in range(KF):
                        nc.tensor.matmul(
                            acc[:],
                            w2s[:, k * D + m * P : k * D + (m + 1) * P],
                            h[:, k * n + c0 : k * n + c0 + csz],
                            start=(k == 0),
                            stop=(k == KF - 1),
                        )
                    last = m == KD - 1 and ci == len(chunks) - 1
                    if not last:
                        nc.scalar.activation(
                            p[:, m * n + c0 : m * n + c0 + csz],
                            acc[:],
                            AF.Exp,
                            bias=b2t[:, m : m + 1],
                        )
                        # sync+scalar are the HWDGE rings (fast completion);
                        # keep outputs off the SWDGE (gpsimd) path
                        eng = nc.sync if m % 2 == 0 else nc.scalar
                        eng.dma_start(
                            outT[m * P : (m + 1) * P, c0 : c0 + csz],
                            p[:, m * n + c0 : m * n + c0 + csz],
                        )
                    else:
                        # halve the final act so its first DMA overlaps the
                        # second half's activation
                        hsz = max(csz // 2, 1)
                        for hi, (h0, hs) in enumerate([(0, hsz), (hsz, csz - hsz)]):
                            if hs <= 0:
                                continue
                            nc.scalar.activation(
                                p[:, m * n + c0 + h0 : m * n + c0 + h0 + hs],
                                acc[:, h0 : h0 + hs],
                                AF.Exp,
                                bias=b2t[:, m : m + 1],
                            )
                            eng = nc.sync if hi == 0 else nc.scalar
                            eng.dma_start(
                                outT[m * P : (m + 1) * P, c0 + h0 : c0 + h0 + hs],
                                p[:, m * n + c0 + h0 : m * n + c0 + h0 + hs],
                            )

    nc.finalize()
    return nc


def _get_nc(n, a_fp8_kd):
    key = (n, a_fp8_kd)
    if key not in _CACHE:
        _CACHE[key] = _build_nc(n, a_fp8_kd)
    return _CACHE[key]


def kernel(x, gate_w, gate_b, w1, b1, w2, b2, top_k, use_bf16=None,
           a_fp8_kd=A_FP8_KD, _trace=False, _tmpdir=None):
    import ml_dtypes

    f8np = ml_dtypes.float8_e4m3

    x = np.asarray(x)
    B, S, _ = x.shape
    T = B * S
    assert int(top_k) == 2
    c = _gating_coeffs(x, gate_w, gate_b)

    x_f = np.ascontiguousarray(x.reshape(T, D).astype(np.float32))
    idx = [np.nonzero(c[:, ex])[0] for ex in (0, 1)]  # tokens per expert
    per_core = max((len(idx[0]) + 3) // 4, (len(idx[1]) + 3) // 4, 1)
    n = ((per_core + P - 1) // P) * P  # padded tokens per core
    KD, KF = D // P, F // P
    MG = F // MGC
    cut = a_fp8_kd * P
    KK = a_fp8_kd // 2
    K16 = KD - a_fp8_kd

    def q8(a):
        return np.ascontiguousarray(np.clip(a, -240, 240).astype(f8np))

    w1 = np.asarray(w1, dtype=np.float32)
    w2 = np.asarray(w2, dtype=np.float32)
    b1 = np.asarray(b1, dtype=np.float32)
    b2 = np.asarray(b2, dtype=np.float32)
    wconv = {}
    for ex in (0, 1):
        # w116[p, mg*K16+k, col] = w1[cut + k*128+p, mg*512+col]
        w116 = np.ascontiguousarray(
            w1[ex][cut:].reshape(K16, P, MG, MGC).transpose(1, 2, 0, 3).reshape(P, MG * K16, MGC)
        ).astype(np.float16)
        # w2p[p, k*D + d] = w2[k*128+p, d]
        w2p = np.ascontiguousarray(
            w2[ex].reshape(KF, P, D).transpose(1, 0, 2).reshape(P, -1)
        ).astype(np.float16)
        ent = {"w1p": w116, "w2p": w2p}
        if KK:
            # w18[p, (mg*KK+kk)*2+pr, col] = w1[kk*256+pr*128+p, mg*512+col]
            ent["w18"] = q8(
                w1[ex][:cut].reshape(KK, 2, P, MG, MGC).transpose(2, 3, 0, 1, 4)
                .reshape(P, MG * KK * 2, MGC)
            )
        wconv[ex] = ent

    in_maps = []
    core_tok = []  # per-core real token ids
    for core in range(NCORES):
        ex = core // 4
        part = core % 4
        ids = idx[ex][part * per_core : (part + 1) * per_core]
        core_tok.append(ids)
        xTc = np.zeros((D, n), dtype=np.float32)
        if len(ids):
            xTc[:, : len(ids)] = x_f[ids].T
        # xs16[p, k, t] = xT[cut + k*128+p, t]
        xs16 = np.ascontiguousarray(
            xTc[cut:].reshape(K16, P, n).transpose(1, 0, 2)
        ).astype(np.float16)
        im = {
            "xs": xs16,
            "b1t": np.ascontiguousarray(b1[ex].reshape(KF, P).T.astype(np.float32)),
            "b2t": np.ascontiguousarray(b2[ex].reshape(KD, P).T.astype(np.float32)),
        }
        im.update(wconv[ex])
        if KK:
            # xs8[p, 2kk+pr, t] = xT[kk*256 + pr*128 + p, t]
            im["xs8"] = q8(
                xTc[:cut].reshape(KK, 2, P, n).transpose(2, 0, 1, 3).reshape(P, KK * 2, n)
            )
        in_maps.append(im)

    nc = _get_nc(n, a_fp8_kd)
    kw = {}
    if _trace:
        kw = {"trace": True, "tmpdir": _tmpdir}
    res = run_bass_kernel_spmd(nc, in_maps, core_ids=list(range(NCORES)), **kw)
    kernel.last_results = res

    out = np.zeros((T, D), dtype=np.float32)
    for core in range(NCORES):
        ids = core_tok[core]
        ex = core // 4
        if len(ids) == 0:
            continue
        pT = res.results[core]["outT"][:, : len(ids)].astype(np.float32)  # [D, n_real]
        s = pT.sum(axis=0)  # softmax denominator per token
        g = c[ids, ex] / s
        out[ids] += (pT * g[None, :]).T
    return out.reshape(B, S, D)


kernel.last_results = None


# revision 15
# speedup vs baseline: 1.1674x; 1.1674x over previous
"""MoE routing kernel for Trainium2 (8 NeuronCores, SPMD).

Math being implemented (faithful to the reference, including its quirks):
  logits = x @ gate_w + gate_b                  # [B,S,E]
  weights = softmax(logits, axis=1)             # softmax over the SEQUENCE axis
  top2 values/indices over experts; only experts 0 and 1 are ever evaluated
  (the reference loops `for ind in range(top_k)` and uses expert `ind`).
  out[t] = c0[t]*eo_0[t] + c1[t]*eo_1[t], where
  eo_e = softmax_D(gelu(x@w1[e]+b1[e]) @ w2[e] + b2[e]) and c_e[t] is the
  top-2 gate weight when expert e is in token t's top-2, else 0.

Sharding: routing + dispatch on host (0.4% of FLOPs). Only tokens whose
top-2 contains expert 0/1 are computed (~25% each). Cores 0-3 handle
expert 0's tokens, cores 4-7 expert 1's. Device computes p = exp(z)
unnormalized in feature-major layout; the softmax division and gate-weight
scaling happen on host during the gather (O(T*D) adds, off the device
critical path).

Device kernel structure (per core, n tokens):
  - all of w1/w2 resident in SBUF (host pre-permuted layouts, ~30 large
    DMAs total); ring FIFO ordering delivers xs -> w1 groups -> w2 so
    phase B's weights never compete with phase A's
  - warmup matmuls on a memset tile so the PE HAM clock ramps to 2.4 GHz
    before real work lands
  - phase A: optionally the first `a_fp8_kd` k-tiles of the contraction run
    as fp8e4m3 DoubleRow matmuls (2x PE rate); the rest run fp16. m-major,
    k-inner, rotating PSUM banks, ACT applies gelu(+b1) -> h (fp16)
  - phase B: fp16; ACT exp(+b2) -> p, DMA'd out as each m-tile completes
"""

import sys

import numpy as np

sys.path.insert(0, "/opt/trn_rl_repo")

import concourse.bacc as bacc  # noqa: E402
import concourse.bass as bass  # noqa: E402
import concourse.tile as tile  # noqa: E402
from concourse import mybir  # noqa: E402
from concourse.bass_utils import run_bass_kernel_spmd  # noqa: E402

P = 128
D = 1024
F = 4096
NCORES = 8
CHUNK = 512  # psum bank free-dim capacity (f32)
MGC = 512    # w1 m-group column width (4 m-tiles)
A_FP8_KD = 4  # k-tiles (of 8) of phase A contraction in fp8 DoubleRow
AF = mybir.ActivationFunctionType

_CACHE = {}


def _gating_coeffs(x, gate_w, gate_b):
    """Host replica of the reference gating. Returns c[T,2] float32 where
    c[:,e] is the gate weight if expert e is in the token's top-2 else 0."""
    B, S, _ = x.shape
    x = np.asarray(x, dtype=np.float32)
    logits = x.reshape(B * S, -1) @ np.asarray(gate_w, dtype=np.float32)
    logits = logits.reshape(B, S, -1) + np.asarray(gate_b, dtype=np.float32)
    # softmax over the sequence axis (axis=1), as in the reference
    m = logits.max(axis=1, keepdims=True)
    e = np.exp(logits - m)
    w = e / e.sum(axis=1, keepdims=True)
    wf = w.reshape(B * S, -1)
    # stable argsort of -w == jax.lax.top_k tie semantics (lower index wins)
    top2 = np.argsort(-wf, axis=-1, kind="stable")[:, :2]
    c = np.zeros((B * S, 2), dtype=np.float32)
    for ex in (0, 1):
        sel = (top2 == ex).any(axis=1)
        c[sel, ex] = wf[sel, ex]
    return c


def _build_nc(n, a_fp8_kd, n_warm=12):
    """Bass program for one core: n tokens (multiple of 128), one expert."""
    dt = mybir.dt
    sdt = dt.float16
    f8 = dt.float8e4
    f32 = dt.float32
    chunks = []
    off = 0
    while off < n:
        sz = min(CHUNK, n - off)
        chunks.append((off, sz))
        off += sz
    KD, KF = D // P, F // P  # 8, 32
    MG = F // MGC            # 8 w1 column groups
    JG = MGC // P            # m-tiles per w1 group
    KK = a_fp8_kd // 2       # DoubleRow k-pairs
    K16 = KD - a_fp8_kd      # fp16 k-tiles in phase A
    DR = mybir.MatmulPerfMode.DoubleRow

    nc = bacc.Bacc()
    # host-prepermuted layouts (see kernel() below)
    if KK:
        x8d = nc.dram_tensor("xs8", [P, KK * 2, n], f8, kind="ExternalInput")
        w18d = nc.dram_tensor("w18", [P, MG * KK * 2, MGC], f8, kind="ExternalInput")
    xd = nc.dram_tensor("xs", [P, K16, n], sdt, kind="ExternalInput")
    w1d = nc.dram_tensor("w1p", [P, MG * K16, MGC], sdt, kind="ExternalInput")
    w2d = nc.dram_tensor("w2p", [P, KF * D], sdt, kind="ExternalInput")
    b1d = nc.dram_tensor("b1t", [P, KF], f32, kind="ExternalInput")
    b2d = nc.dram_tensor("b2t", [P, KD], f32, kind="ExternalInput")
    outT = nc.dram_tensor("outT", [D, n], sdt, kind="ExternalOutput")

    with tile.TileContext(nc) as tc:
        with (
            tc.tile_pool(name="const", bufs=1) as const,
            tc.tile_pool(name="acts", bufs=1) as acts,
            tc.tile_pool(name="ps", bufs=8, space="PSUM") as ps,
        ):
            # ---- input DMAs: few, large, spread across sequencers ----
            # Ring rates ~140 GB/s each on sync/gpsimd, aggregate ~285 GB/s.
            # FIFO per ring orders delivery: xs first (m=0 needs ALL k-tiles),
            # then w1 groups in consumption order, then w2 (needed only for
            # phase B, naturally deferred behind w1 by ring FIFO).
            xs16 = acts.tile([P, K16, n], sdt)
            if KK:
                xs8 = acts.tile([P, KK * 2, n], f8)
                nc.sync.dma_start(xs8[:, :, :], x8d[:, :, :])
                hk = K16 // 2
                nc.gpsimd.dma_start(xs16[:, :hk, :], xd[:, :hk, :])
                nc.sync.dma_start(xs16[:, hk:, :], xd[:, hk:, :])
            else:
                for q in range(4):
                    eng = nc.sync if q % 2 == 0 else nc.gpsimd
                    k0, k1 = q * K16 // 4, (q + 1) * K16 // 4
                    eng.dma_start(xs16[:, k0:k1, :], xd[:, k0:k1, :])
            b1t = const.tile([P, KF], f32)
            nc.scalar.dma_start(b1t[:], b1d[:])
            b2t = const.tile([P, KD], f32)
            nc.scalar.dma_start(b2t[:], b2d[:])

            if KK:
                w18 = acts.tile([P, MG * KK * 2, MGC], f8)
            w116 = acts.tile([P, MG * K16, MGC], sdt)
            for mg in range(MG):
                eng, eng2 = (nc.sync, nc.gpsimd) if mg % 2 == 0 else (nc.gpsimd, nc.sync)
                if mg == 0:
                    # m-tile 0's weight columns first: shrinks the critical
                    # DMA set gating the first real matmul
                    if KK:
                        eng.dma_start(w18[:, : KK * 2, :P], w18d[:, : KK * 2, :P])
                    eng2.dma_start(w116[:, :K16, :P], w1d[:, :K16, :P])
                    if KK:
                        eng2.dma_start(w18[:, : KK * 2, P:], w18d[:, : KK * 2, P:])
                    eng.dma_start(w116[:, :K16, P:], w1d[:, :K16, P:])
                    continue
                if KK:
                    eng.dma_start(
                        w18[:, mg * KK * 2 : (mg + 1) * KK * 2, :],
                        w18d[:, mg * KK * 2 : (mg + 1) * KK * 2, :],
                    )
                eng2.dma_start(
                    w116[:, mg * K16 : (mg + 1) * K16, :],
                    w1d[:, mg * K16 : (mg + 1) * K16, :],
                )
            w2s = acts.tile([P, KF * D], sdt)
            NW2 = 8
            for q in range(NW2):
                eng = nc.sync if q % 2 == 0 else nc.gpsimd
                sz = KF * D // NW2
                eng.dma_start(w2s[:, q * sz : (q + 1) * sz], w2d[:, q * sz : (q + 1) * sz])

            h = acts.tile([P, KF * n], sdt)
            p = acts.tile([P, KD * n], sdt)

            # ---- HAM warmup: matmuls gated only on a cheap DVE memset ----
            # small tile -> fast memset -> first matmul right after the PE
            # preamble; 256-wide warmups at double count span the same time
            garb = const.tile([P, 2 * P], sdt)
            nc.vector.memset(garb[:], 1.0)
            warm_ps = ps.tile([P, CHUNK], f32, tag="ps", name="warm")
            for _ in range(2 * n_warm):
                nc.tensor.matmul(
                    warm_ps[:, : 2 * P], garb[:, :P], garb[:, : 2 * P],
                    start=True, stop=True,
                )

            # ---- Phase A: h = gelu(w1.T @ x.T + b1), m-major k-inner ----
            for m in range(KF):
                mg, j = divmod(m, JG)
                for ci, (c0, csz) in enumerate(chunks):
                    acc = ps.tile([P, csz], f32, tag="ps", name=f"pa_{m}_{ci}")
                    for kk in range(KK):
                        i0 = (mg * KK + kk) * 2
                        nc.tensor.matmul(
                            acc[:],
                            w18[:, i0 : i0 + 2, j * P : (j + 1) * P],
                            xs8[:, 2 * kk : 2 * kk + 2, c0 : c0 + csz],
                            start=(kk == 0),
                            stop=False,
                            perf_mode=DR,
                        )
                    for k in range(K16):
                        nc.tensor.matmul(
                            acc[:],
                            w116[:, mg * K16 + k, j * P : (j + 1) * P],
                            xs16[:, k, c0 : c0 + csz],
                            start=(KK == 0 and k == 0),
                            stop=(k == K16 - 1),
                        )
                    nc.scalar.activation(
                        h[:, m * n + c0 : m * n + c0 + csz],
                        acc[:],
                        AF.Gelu,
                        bias=b1t[:, m : m + 1],
                    )

            # ---- Phase B: p = exp(w2.T @ h + b2); DMA out as computed ----
            for m in range(KD):
                for ci, (c0, csz) in enumerate(chunks):
                    acc = ps.tile([P, csz], f32, tag="ps", name=f"pb_{m}_{ci}")
                    for k in range(KF):
                        nc.tensor.matmul(
                            acc[:],
                            w2s[:, k * D + m * P : k * D + (m + 1) * P],
                            h[:, k * n + c0 : k * n + c0 + csz],
                            start=(k == 0),
                            stop=(k == KF - 1),
                        )
                    last = m == KD - 1 and ci == len(chunks) - 1
                    if not last:
                        nc.scalar.activation(
                            p[:, m * n + c0 : m * n + c0 + csz],
                            acc[:],
                            AF.Exp,
                            bias=b2t[:, m : m + 1],
                        )
                        # sync+scalar are the HWDGE rings (fast completion);
                        # keep outputs off the SWDGE (gpsimd) path
                        eng = nc.sync if m % 2 == 0 else nc.scalar
                        eng.dma_start(
                            outT[m * P : (m + 1) * P, c0 : c0 + csz],
                            p[:, m * n + c0 : m * n + c0 + csz],
                        )
                    else:
                        # halve the final act so its first DMA overlaps the
                        # second half's activation
                        hsz = max(csz // 2, 1)
                        for hi, (h0, hs) in enumerate([(0, hsz), (hsz, csz - hsz)]):
                            if hs <= 0:
                                continue
                            nc.scalar.activation(
                                p[:, m * n + c0 + h0 : m * n + c0 + h0 + hs],
                                acc[:, h0 : h0 + hs],
                                AF.Exp,
                                bias=b2t[:, m : m + 1],
                            )
                            eng = nc.sync if hi == 0 else nc.scalar
                            eng.dma_start(
                                outT[m * P : (m + 1) * P, c0 + h0 : c0 + h0 + hs],
                                p[:, m * n + c0 + h0 : m * n + c0 + h0 + hs],
                            )

    nc.finalize()
    return nc


def _get_nc(n, a_fp8_kd):
    key = (n, a_fp8_kd)
    if key not in _CACHE:
        _CACHE[key] = _build_nc(n, a_fp8_kd)
    return _CACHE[key]


def kernel(x, gate_w, gate_b, w1, b1, w2, b2, top_k, use_bf16=None,
           a_fp8_kd=A_FP8_KD, _trace=False, _tmpdir=None):
    import ml_dtypes

    f8np = ml_dtypes.float8_e4m3

    x = np.asarray(x)
    B, S, _ = x.shape
    T = B * S
    assert int(top_k) == 2
    c = _gating_coeffs(x, gate_w, gate_b)

    x_f = np.ascontiguousarray(x.reshape(T, D).astype(np.float32))
    idx = [np.nonzero(c[:, ex])[0] for ex in (0, 1)]  # tokens per expert
    per_core = max((len(idx[0]) + 3) // 4, (len(idx[1]) + 3) // 4, 1)
    n = ((per_core + P - 1) // P) * P  # padded tokens per core
    KD, KF = D // P, F // P
    MG = F // MGC
    cut = a_fp8_kd * P
    KK = a_fp8_kd // 2
    K16 = KD - a_fp8_kd

    def q8(a):
        return np.ascontiguousarray(np.clip(a, -240, 240).astype(f8np))

    w1 = np.asarray(w1, dtype=np.float32)
    w2 = np.asarray(w2, dtype=np.float32)
    b1 = np.asarray(b1, dtype=np.float32)
    b2 = np.asarray(b2, dtype=np.float32)
    wconv = {}
    for ex in (0, 1):
        # w116[p, mg*K16+k, col] = w1[cut + k*128+p, mg*512+col]
        w116 = np.ascontiguousarray(
            w1[ex][cut:].reshape(K16, P, MG, MGC).transpose(1, 2, 0, 3).reshape(P, MG * K16, MGC)
        ).astype(np.float16)
        # w2p[p, k*D + d] = w2[k*128+p, d]
        w2p = np.ascontiguousarray(
            w2[ex].reshape(KF, P, D).transpose(1, 0, 2).reshape(P, -1)
        ).astype(np.float16)
        ent = {"w1p": w116, "w2p": w2p}
        if KK:
            # w18[p, (mg*KK+kk)*2+pr, col] = w1[kk*256+pr*128+p, mg*512+col]
            ent["w18"] = q8(
                w1[ex][:cut].reshape(KK, 2, P, MG, MGC).transpose(2, 3, 0, 1, 4)
                .reshape(P, MG * KK * 2, MGC)
            )
        wconv[ex] = ent

    in_maps = []
    core_tok = []  # per-core real token ids
    for core in range(NCORES):
        ex = core // 4
        part = core % 4
        ids = idx[ex][part * per_core : (part + 1) * per_core]
        core_tok.append(ids)
        xTc = np.zeros((D, n), dtype=np.float32)
        if len(ids):
            xTc[:, : len(ids)] = x_f[ids].T
        # xs16[p, k, t] = xT[cut + k*128+p, t]
        xs16 = np.ascontiguousarray(
            xTc[cut:].reshape(K16, P, n).transpose(1, 0, 2)
        ).astype(np.float16)
        im = {
            "xs": xs16,
            "b1t": np.ascontiguousarray(b1[ex].reshape(KF, P).T.astype(np.float32)),
            "b2t": np.ascontiguousarray(b2[ex].reshape(KD, P).T.astype(np.float32)),
        }
        im.update(wconv[ex])
        if KK:
            # xs8[p, 2kk+pr, t] = xT[kk*256 + pr*128 + p, t]
            im["xs8"] = q8(
                xTc[:cut].reshape(KK, 2, P, n).transpose(2, 0, 1, 3).reshape(P, KK * 2, n)
            )
        in_maps.append(im)

    nc = _get_nc(n, a_fp8_kd)
    kw = {}
    if _trace:
        kw = {"trace": True, "tmpdir": _tmpdir}
    res = run_bass_kernel_spmd(nc, in_maps, core_ids=list(range(NCORES)), **kw)
    kernel.last_results = res

    out = np.zeros((T, D), dtype=np.float32)
    for core in range(NCORES):
        ids = core_tok[core]
        ex = core // 4
        if len(ids) == 0:
            continue
        pT = res.results[core]["outT"][:, : len(ids)].astype(np.float32)  # [D, n_real]
        s = pT.sum(axis=0)  # softmax denominator per token
        g = c[ids, ex] / s
        out[ids] += (pT * g[None, :]).T
    return out.reshape(B, S, D)


kernel.last_results = None


# revision 17
# speedup vs baseline: 1.1919x; 1.0209x over previous
"""MoE routing kernel for Trainium2 (8 NeuronCores, SPMD).

Math being implemented (faithful to the reference, including its quirks):
  logits = x @ gate_w + gate_b                  # [B,S,E]
  weights = softmax(logits, axis=1)             # softmax over the SEQUENCE axis
  top2 values/indices over experts; only experts 0 and 1 are ever evaluated
  (the reference loops `for ind in range(top_k)` and uses expert `ind`).
  out[t] = c0[t]*eo_0[t] + c1[t]*eo_1[t], where
  eo_e = softmax_D(gelu(x@w1[e]+b1[e]) @ w2[e] + b2[e]) and c_e[t] is the
  top-2 gate weight when expert e is in token t's top-2, else 0.

Sharding: routing + dispatch on host (0.4% of FLOPs). Only tokens whose
top-2 contains expert 0/1 are computed (~25% each). Cores 0-3 handle
expert 0's tokens, cores 4-7 expert 1's. Device computes p = exp(z)
unnormalized in feature-major layout; the softmax division and gate-weight
scaling happen on host during the gather (O(T*D) adds, off the device
critical path).

Device kernel structure (per core, n tokens):
  - all of w1/w2 resident in SBUF (host pre-permuted layouts, ~30 large
    DMAs total); ring FIFO ordering delivers xs -> w1 groups -> w2 so
    phase B's weights never compete with phase A's
  - warmup matmuls on a memset tile so the PE HAM clock ramps to 2.4 GHz
    before real work lands
  - phase A: optionally the first `a_fp8_kd` k-tiles of the contraction run
    as fp8e4m3 DoubleRow matmuls (2x PE rate); the rest run fp16. m-major,
    k-inner, rotating PSUM banks, ACT applies gelu(+b1) -> h (fp16)
  - phase B: fp16; ACT exp(+b2) -> p, DMA'd out as each m-tile completes
"""

import sys

import numpy as np

sys.path.insert(0, "/opt/trn_rl_repo")

import concourse.bacc as bacc  # noqa: E402
import concourse.tile as tile  # noqa: E402
from concourse import mybir  # noqa: E402
from concourse.bass_utils import run_bass_kernel_spmd  # noqa: E402

P = 128
D = 1024
F = 4096
NCORES = 8
CHUNK = 512  # psum bank free-dim capacity (f32)
MGC = 512    # w1 m-group column width (4 m-tiles)
A_FP8_KD = 4  # k-tiles (of 8) of phase A contraction in fp8 DoubleRow
AF = mybir.ActivationFunctionType

_CACHE = {}


def _gating_coeffs(x, gate_w, gate_b):
    """Host replica of the reference gating. Returns c[T,2] float32 where
    c[:,e] is the gate weight if expert e is in the token's top-2 else 0."""
    B, S, _ = x.shape
    x = np.asarray(x, dtype=np.float32)
    logits = x.reshape(B * S, -1) @ np.asarray(gate_w, dtype=np.float32)
    logits = logits.reshape(B, S, -1) + np.asarray(gate_b, dtype=np.float32)
    # softmax over the sequence axis (axis=1), as in the reference
    m = logits.max(axis=1, keepdims=True)
    e = np.exp(logits - m)
    w = e / e.sum(axis=1, keepdims=True)
    wf = w.reshape(B * S, -1)
    # stable argsort of -w == jax.lax.top_k tie semantics (lower index wins)
    top2 = np.argsort(-wf, axis=-1, kind="stable")[:, :2]
    c = np.zeros((B * S, 2), dtype=np.float32)
    for ex in (0, 1):
        sel = (top2 == ex).any(axis=1)
        c[sel, ex] = wf[sel, ex]
    return c


def _build_nc(n, a_fp8_kd, n_warm=12):
    """Bass program for one core: n tokens (multiple of 128), one expert."""
    dt = mybir.dt
    sdt = dt.float16
    f8 = dt.float8e4
    f32 = dt.float32
    chunks = []
    off = 0
    while off < n:
        sz = min(CHUNK, n - off)
        chunks.append((off, sz))
        off += sz
    KD, KF = D // P, F // P  # 8, 32
    MG = F // MGC            # 8 w1 column groups
    JG = MGC // P            # m-tiles per w1 group
    KK = a_fp8_kd // 2       # DoubleRow k-pairs
    K16 = KD - a_fp8_kd      # fp16 k-tiles in phase A
    DR = mybir.MatmulPerfMode.DoubleRow

    nc = bacc.Bacc()
    # host-prepermuted layouts (see kernel() below)
    if KK:
        x8d = nc.dram_tensor("xs8", [P, KK * 2, n], f8, kind="ExternalInput")
        w18d = nc.dram_tensor("w18", [P, MG * KK * 2, MGC], f8, kind="ExternalInput")
    xd = nc.dram_tensor("xs", [P, K16, n], sdt, kind="ExternalInput")
    w1d = nc.dram_tensor("w1p", [P, MG * K16, MGC], sdt, kind="ExternalInput")
    w2d = nc.dram_tensor("w2p", [P, KF * D], sdt, kind="ExternalInput")
    b1d = nc.dram_tensor("b1t", [P, KF], f32, kind="ExternalInput")
    b2d = nc.dram_tensor("b2t", [P, KD], f32, kind="ExternalInput")
    outT = nc.dram_tensor("outT", [D, n], sdt, kind="ExternalOutput")

    with tile.TileContext(nc) as tc:
        with (
            tc.tile_pool(name="const", bufs=1) as const,
            tc.tile_pool(name="acts", bufs=1) as acts,
            tc.tile_pool(name="ps", bufs=8, space="PSUM") as ps,
        ):
            # ---- input DMAs: few, large, spread across sequencers ----
            # Ring rates ~140 GB/s each on sync/gpsimd, aggregate ~285 GB/s.
            # FIFO per ring orders delivery: xs first (m=0 needs ALL k-tiles),
            # then w1 groups in consumption order, then w2 (needed only for
            # phase B, naturally deferred behind w1 by ring FIFO).
            xs16 = acts.tile([P, K16, n], sdt)
            if KK:
                xs8 = acts.tile([P, KK * 2, n], f8)
                nc.sync.dma_start(xs8[:, :, :], x8d[:, :, :])
                hk = K16 // 2
                nc.gpsimd.dma_start(xs16[:, :hk, :], xd[:, :hk, :])
                nc.sync.dma_start(xs16[:, hk:, :], xd[:, hk:, :])
            else:
                for q in range(4):
                    eng = nc.sync if q % 2 == 0 else nc.gpsimd
                    k0, k1 = q * K16 // 4, (q + 1) * K16 // 4
                    eng.dma_start(xs16[:, k0:k1, :], xd[:, k0:k1, :])
            b1t = const.tile([P, KF], f32)
            nc.scalar.dma_start(b1t[:], b1d[:])
            b2t = const.tile([P, KD], f32)
            nc.scalar.dma_start(b2t[:], b2d[:])

            if KK:
                w18 = acts.tile([P, MG * KK * 2, MGC], f8)
            w116 = acts.tile([P, MG * K16, MGC], sdt)
            for mg in range(MG):
                eng, eng2 = (nc.sync, nc.gpsimd) if mg % 2 == 0 else (nc.gpsimd, nc.sync)
                if mg == 0:
                    # m-tile 0's weight columns first: shrinks the critical
                    # DMA set gating the first real matmul
                    if KK:
                        eng.dma_start(w18[:, : KK * 2, :P], w18d[:, : KK * 2, :P])
                    eng2.dma_start(w116[:, :K16, :P], w1d[:, :K16, :P])
                    if KK:
                        eng2.dma_start(w18[:, : KK * 2, P:], w18d[:, : KK * 2, P:])
                    eng.dma_start(w116[:, :K16, P:], w1d[:, :K16, P:])
                    continue
                if KK:
                    eng.dma_start(
                        w18[:, mg * KK * 2 : (mg + 1) * KK * 2, :],
                        w18d[:, mg * KK * 2 : (mg + 1) * KK * 2, :],
                    )
                eng2.dma_start(
                    w116[:, mg * K16 : (mg + 1) * K16, :],
                    w1d[:, mg * K16 : (mg + 1) * K16, :],
                )
            w2s = acts.tile([P, KF * D], sdt)
            NW2 = 8
            for q in range(NW2):
                eng = nc.sync if q % 2 == 0 else nc.gpsimd
                sz = KF * D // NW2
                eng.dma_start(w2s[:, q * sz : (q + 1) * sz], w2d[:, q * sz : (q + 1) * sz])

            h = acts.tile([P, KF * n], sdt)
            p = acts.tile([P, KD * n], sdt)

            # ---- HAM warmup: matmuls gated only on a cheap DVE memset ----
            # sized to keep the PE busy (HAM warm) until the first xs/w1
            # DMAs complete at ~13us; ending early lets HAM re-throttle
            garb = const.tile([P, P + CHUNK], sdt)
            nc.vector.memset(garb[:], 1.0)
            warm_ps = ps.tile([P, CHUNK], f32, tag="ps", name="warm")
            for _ in range(n_warm):
                nc.tensor.matmul(
                    warm_ps[:], garb[:, :P], garb[:, P : P + CHUNK],
                    start=True, stop=True,
                )

            # ---- Phase A: h = gelu(w1.T @ x.T + b1), m-major k-inner ----
            for m in range(KF):
                mg, j = divmod(m, JG)
                for ci, (c0, csz) in enumerate(chunks):
                    acc = ps.tile([P, csz], f32, tag="ps", name=f"pa_{m}_{ci}")
                    for kk in range(KK):
                        i0 = (mg * KK + kk) * 2
                        nc.tensor.matmul(
                            acc[:],
                            w18[:, i0 : i0 + 2, j * P : (j + 1) * P],
                            xs8[:, 2 * kk : 2 * kk + 2, c0 : c0 + csz],
                            start=(kk == 0),
                            stop=False,
                            perf_mode=DR,
                        )
                    for k in range(K16):
                        nc.tensor.matmul(
                            acc[:],
                            w116[:, mg * K16 + k, j * P : (j + 1) * P],
                            xs16[:, k, c0 : c0 + csz],
                            start=(KK == 0 and k == 0),
                            stop=(k == K16 - 1),
                        )
                    nc.scalar.activation(
                        h[:, m * n + c0 : m * n + c0 + csz],
                        acc[:],
                        AF.Gelu,
                        bias=b1t[:, m : m + 1],
                    )

            # ---- Phase B: p = exp(w2.T @ h + b2); DMA out as computed ----
            for m in range(KD):
                for ci, (c0, csz) in enumerate(chunks):
                    acc = ps.tile([P, csz], f32, tag="ps", name=f"pb_{m}_{ci}")
                    for k in range(KF):
                        nc.tensor.matmul(
                            acc[:],
                            w2s[:, k * D + m * P : k * D + (m + 1) * P],
                            h[:, k * n + c0 : k * n + c0 + csz],
                            start=(k == 0),
                            stop=(k == KF - 1),
                        )
                    last = m == KD - 1 and ci == len(chunks) - 1
                    if not last:
                        nc.scalar.activation(
                            p[:, m * n + c0 : m * n + c0 + csz],
                            acc[:],
                            AF.Exp,
                            bias=b2t[:, m : m + 1],
                        )
                        # sync+scalar are the HWDGE rings (fast completion);
                        # keep outputs off the SWDGE (gpsimd) path
                        eng = nc.sync if m % 2 == 0 else nc.scalar
                        eng.dma_start(
                            outT[m * P : (m + 1) * P, c0 : c0 + csz],
                            p[:, m * n + c0 : m * n + c0 + csz],
                        )
                    else:
                        # halve the final act so its first DMA overlaps the
                        # second half's activation
                        hsz = max(csz // 2, 1)
                        for hi, (h0, hs) in enumerate([(0, hsz), (hsz, csz - hsz)]):
                            if hs <= 0:
                                continue
                            nc.scalar.activation(
                                p[:, m * n + c0 + h0 : m * n + c0 + h0 + hs],
                                acc[:, h0 : h0 + hs],
                                AF.Exp,
                                bias=b2t[:, m : m + 1],
                            )
                            eng = nc.sync if hi == 0 else nc.scalar
                            eng.dma_start(
                                outT[m * P : (m + 1) * P, c0 + h0 : c0 + h0 + hs],
                                p[:, m * n + c0 + h0 : m * n + c0 + h0 + hs],
                            )

    nc.finalize()
    return nc


def _get_nc(n, a_fp8_kd):
    key = (n, a_fp8_kd)
    if key not in _CACHE:
        _CACHE[key] = _build_nc(n, a_fp8_kd)
    return _CACHE[key]


def kernel(x, gate_w, gate_b, w1, b1, w2, b2, top_k, use_bf16=None,
           a_fp8_kd=A_FP8_KD, _trace=False, _tmpdir=None):
    import ml_dtypes

    f8np = ml_dtypes.float8_e4m3

    x = np.asarray(x)
    B, S, _ = x.shape
    T = B * S
    assert int(top_k) == 2
    c = _gating_coeffs(x, gate_w, gate_b)

    x_f = np.ascontiguousarray(x.reshape(T, D).astype(np.float32))
    idx = [np.nonzero(c[:, ex])[0] for ex in (0, 1)]  # tokens per expert
    per_core = max((len(idx[0]) + 3) // 4, (len(idx[1]) + 3) // 4, 1)
    n = ((per_core + P - 1) // P) * P  # padded tokens per core
    KD, KF = D // P, F // P
    MG = F // MGC
    cut = a_fp8_kd * P
    KK = a_fp8_kd // 2
    K16 = KD - a_fp8_kd

    def q8(a):
        return np.ascontiguousarray(np.clip(a, -240, 240).astype(f8np))

    w1 = np.asarray(w1, dtype=np.float32)
    w2 = np.asarray(w2, dtype=np.float32)
    b1 = np.asarray(b1, dtype=np.float32)
    b2 = np.asarray(b2, dtype=np.float32)
    wconv = {}
    for ex in (0, 1):
        # w116[p, mg*K16+k, col] = w1[cut + k*128+p, mg*512+col]
        w116 = np.ascontiguousarray(
            w1[ex][cut:].reshape(K16, P, MG, MGC).transpose(1, 2, 0, 3).reshape(P, MG * K16, MGC)
        ).astype(np.float16)
        # w2p[p, k*D + d] = w2[k*128+p, d]
        w2p = np.ascontiguousarray(
            w2[ex].reshape(KF, P, D).transpose(1, 0, 2).reshape(P, -1)
        ).astype(np.float16)
        ent = {"w1p": w116, "w2p": w2p}
        if KK:
            # w18[p, (mg*KK+kk)*2+pr, col] = w1[kk*256+pr*128+p, mg*512+col]
            ent["w18"] = q8(
                w1[ex][:cut].reshape(KK, 2, P, MG, MGC).transpose(2, 3, 0, 1, 4)
                .reshape(P, MG * KK * 2, MGC)
            )
        wconv[ex] = ent

    in_maps = []
    core_tok = []  # per-core real token ids
    for core in range(NCORES):
        ex = core // 4
        part = core % 4
        ids = idx[ex][part * per_core : (part + 1) * per_core]
        core_tok.append(ids)
        xTc = np.zeros((D, n), dtype=np.float32)
        if len(ids):
            xTc[:, : len(ids)] = x_f[ids].T
        # xs16[p, k, t] = xT[cut + k*128+p, t]
        xs16 = np.ascontiguousarray(
            xTc[cut:].reshape(K16, P, n).transpose(1, 0, 2)
        ).astype(np.float16)
        im = {
            "xs": xs16,
            "b1t": np.ascontiguousarray(b1[ex].reshape(KF, P).T.astype(np.float32)),
            "b2t": np.ascontiguousarray(b2[ex].reshape(KD, P).T.astype(np.float32)),
        }
        im.update(wconv[ex])
        if KK:
            # xs8[p, 2kk+pr, t] = xT[kk*256 + pr*128 + p, t]
            im["xs8"] = q8(
                xTc[:cut].reshape(KK, 2, P, n).transpose(2, 0, 1, 3).reshape(P, KK * 2, n)
            )
        in_maps.append(im)

    nc = _get_nc(n, a_fp8_kd)
    kw = {}
    if _trace:
        kw = {"trace": True, "tmpdir": _tmpdir}
    res = run_bass_kernel_spmd(nc, in_maps, core_ids=list(range(NCORES)), **kw)
    kernel.last_results = res

    out = np.zeros((T, D), dtype=np.float32)
    for core in range(NCORES):
        ids = core_tok[core]
        ex = core // 4
        if len(ids) == 0:
            continue
        pT = res.results[core]["outT"][:, : len(ids)].astype(np.float32)  # [D, n_real]
        s = pT.sum(axis=0)  # softmax denominator per token
        g = c[ids, ex] / s
        out[ids] += (pT * g[None, :]).T
    return out.reshape(B, S, D)


kernel.last_results = None
